# revision 3
# baseline (speedup 1.0000x reference)
"""Causal single-head attention (B=4, T=4096, C=1024, H=64) on 8 TRN2 cores.

Sharding: core = 2*b + h  (b = batch, h = kv-parity).  Each core computes,
for ALL queries of its batch, the partial softmax numerator and denominator
over the kv chunks (128 rows each) whose chunk index has parity h.  This
makes the per-core program identical across cores (SPMD requirement) and
perfectly load-balanced; the host combines partials:
    out = (num0 + num1) / (den0 + den1).

The host passes x[b].T with each 512-query superblock's four 128-column
chunks permuted parity-major (chunks [h, 2+h, 1-h, 3-h]), so the kv-parity
columns of every superblock sit at positions 0:256.  K/V projections then
read the resident xq tiles directly (no second x input), and the program
stays identical across cores; causal masks (host-built, per-core) and
host-side row unpermutation absorb the data-dependence.

Device data flow (per core):
  xq pair tiles [128, 2, CC, 512] (two superblocks per tile)
  kvT [128, 512] per kv-super (rows 0:64 kT, 64:128 vT);  V row-chunks via
      PE transpose.  qT [64, 512] per q-super.
  S^T pair [128kv, 2, 512q] = kT.T @ qT    (PE, contraction H=64)
  P^T = exp(S^T/32) (single ACT op per pair) * causal mask (DVE, last pair)
  outT [65, 512q] += Vaug.T @ P^T          (Vaug = [V | ones], PE)
  out rows via PE transpose of outT 128-col blocks.
"""

import numpy as np
import ml_dtypes

import concourse.bass as bass
import concourse.bacc as bacc
import concourse.tile as tile
from concourse import mybir
from concourse.bass_utils import run_bass_kernel_spmd

F32 = mybir.dt.float32
BF16 = mybir.dt.bfloat16

B = 4
C = 1024
H = 64
SUP = 512          # query superblock width
KC = 128           # kv chunk
CC = C // 128      # contraction chunks


def chunk_perm(h):
    """Within-superblock 128-col chunk order: parity-h chunks first."""
    return [h, 2 + h, 1 - h, 3 - h]


def build_nc(T=4096, pt_bufs=3, att_order=None, reps=1):
    n_sup = T // SUP
    n_pair = n_sup // 2
    n_kv_sup = n_pair          # one kv-super (512 parity cols) per pair
    scale = float(C) ** -0.5

    nc = bacc.Bacc(None, target_bir_lowering=False)
    xq_d = nc.dram_tensor("xq", [C, T], BF16, kind="ExternalInput")
    wq_d = nc.dram_tensor("wq", [C, H], BF16, kind="ExternalInput")
    wkv_d = nc.dram_tensor("wkv", [C, 2 * H], BF16, kind="ExternalInput")
    bq_d = nc.dram_tensor("bq", [H, 1], F32, kind="ExternalInput")
    bkv_d = nc.dram_tensor("bkv", [2 * H, 1], F32, kind="ExternalInput")
    mask_d = nc.dram_tensor("mask", [128, 2, SUP], BF16, kind="ExternalInput")
    idf_d = nc.dram_tensor("identf", [128, 128], F32, kind="ExternalInput")
    idb_d = nc.dram_tensor("identb", [128, 128], BF16, kind="ExternalInput")
    out_d = nc.dram_tensor("out", [T, H + 1], F32, kind="ExternalOutput")

    with tile.TileContext(nc) as tc:
        with (
            tc.tile_pool(name="consts", bufs=1) as consts,
            tc.tile_pool(name="pers", bufs=1) as pers,
            tc.tile_pool(name="pt", bufs=pt_bufs) as ptp,
            tc.tile_pool(name="ot", bufs=2) as otsb,
            tc.tile_pool(name="stg", bufs=2) as stgp,
            tc.tile_pool(name="proj", bufs=2, space="PSUM") as projp,
            tc.tile_pool(name="spsum", bufs=2, space="PSUM") as sp,
            tc.tile_pool(name="otp", bufs=1, space="PSUM") as otp,
            tc.tile_pool(name="tpp", bufs=1, space="PSUM") as tpp,
        ):
            # [p, pair, i, cc, t]
            xq_r = xq_d.rearrange("(cc p) (pr i t) -> p pr i cc t",
                                  p=128, i=2, t=SUP)

            # critical-path consts first, one per ring
            wkv_sb = consts.tile([128, CC, 2 * H], BF16)
            nc.sync.dma_start(wkv_sb[:], wkv_d.rearrange("(cc p) m -> p cc m", p=128))
            wq_sb = consts.tile([128, CC, H], BF16)
            nc.scalar.dma_start(wq_sb[:], wq_d.rearrange("(cc p) m -> p cc m", p=128))

            # one resident tile per (pair, c-chunk) for fine-grained deps;
            # pair 0 loads next (kv-super 0 + attention 0/1 gate startup)
            xq_tiles = {}

            def load_pair(pr):
                for c0 in range(CC):
                    t_ = consts.tile([128, 2, SUP], BF16, tag=f"xq{pr}_{c0}")
                    eng = nc.sync if (pr + c0) % 2 == 0 else nc.scalar
                    eng.dma_start(t_[:], xq_r[:, pr, :, c0, :])
                    xq_tiles[pr, c0] = t_

            load_pair(0)
            bkv_sb = consts.tile([2 * H, 1], F32)
            nc.sync.dma_start(bkv_sb[:], bkv_d[:])
            bq_sb = consts.tile([H, 1], F32)
            nc.scalar.dma_start(bq_sb[:], bq_d[:])
            idb_sb = consts.tile([128, 128], BF16)
            nc.sync.dma_start(idb_sb[:], idb_d[:])
            mask_sb = consts.tile([128, 2, SUP], BF16)
            nc.scalar.dma_start(mask_sb[:], mask_d[:])
            idf_sb = consts.tile([128, 128], F32)
            nc.scalar.dma_start(idf_sb[:], idf_d[:])
            for pr in range(1, n_pair):
                load_pair(pr)

            kvT = {}
            vaug = {}

            def emit_kv_super(s):
                ps = projp.tile([128, SUP], F32, tag="proj")
                for c in range(CC):
                    nc.tensor.matmul(ps[:], wkv_sb[:, c, :],
                                     xq_tiles[s, c][:, :, 0:2 * KC],
                                     start=(c == 0), stop=(c == CC - 1))
                kv_sb = pers.tile([128, SUP], BF16, tag=f"kvT{s}")
                nc.vector.tensor_scalar_add(kv_sb[:], ps[:], bkv_sb[:])
                kvT[s] = kv_sb
                # V row chunks via PE transpose of the vT half (parts 64:128)
                for j in range(SUP // KC):
                    va = pers.tile([128, H + 1], BF16, tag=f"vaug{s * 4 + j}")
                    nc.vector.memset(va[:, H:H + 1], 1.0)
                    tp = tpp.tile([128, 128], BF16, tag="tp")
                    nc.tensor.transpose(tp[:, 0:H], kv_sb[64:128, j * KC:(j + 1) * KC],
                                        idb_sb[64:128, 64:128])
                    nc.vector.tensor_copy(va[:, 0:H], tp[:, 0:H])
                    vaug[s * 4 + j] = va

            def emit_attention_super(sg):
                psq = projp.tile([H, SUP], F32, tag="proj")
                for c in range(CC):
                    nc.tensor.matmul(psq[:], wq_sb[:, c, :],
                                     xq_tiles[sg // 2, c][:, sg % 2, :],
                                     start=(c == 0), stop=(c == CC - 1))
                qT = pers.tile([H, SUP], BF16, tag=f"qT{sg}")
                nc.vector.tensor_scalar_add(qT[:], psq[:], bq_sb[:])

                n_loc = 2 * (sg + 1)
                ot_ps = otp.tile([H + 1, SUP], F32)
                for kp in range(n_loc // 2):
                    s_ps = sp.tile([128, 2, SUP], F32)
                    for i in range(2):
                        k = 2 * kp + i
                        skv, off = k // 4, (k % 4) * KC
                        nc.tensor.matmul(s_ps[:, i, :],
                                         kvT[skv][0:64, off:off + KC], qT[:],
                                         start=True, stop=True)
                    pt = ptp.tile([128, 2, SUP], BF16)
                    nc.scalar.activation(pt[:], s_ps[:],
                                         mybir.ActivationFunctionType.Exp,
                                         scale=scale)
                    if kp == n_loc // 2 - 1:
                        nc.vector.tensor_mul(pt[:], pt[:], mask_sb[:])
                    for i in range(2):
                        k = 2 * kp + i
                        nc.tensor.matmul(ot_ps[:], vaug[k][:], pt[:, i, :],
                                         start=(k == 0), stop=(k == n_loc - 1))

                ot_s = otsb.tile([H + 1, SUP], F32)
                nc.vector.tensor_copy(ot_s[:], ot_ps[:])
                stg = stgp.tile([128, SUP // KC, H + 1], F32)
                for qb in range(SUP // KC):
                    tp = tpp.tile([128, 128], F32, tag="tp")
                    nc.tensor.transpose(tp[:, 0:H + 1], ot_s[:, qb * KC:(qb + 1) * KC],
                                        idf_sb[0:H + 1, 0:H + 1])
                    nc.vector.tensor_copy(stg[:, qb, :], tp[:, 0:H + 1])
                nc.sync.dma_start(
                    out_d[sg * SUP:(sg + 1) * SUP, :]
                    .rearrange("(qb p) n -> p qb n", p=128),
                    stg[:])

            for _rep in range(reps):
                if _rep > 0:
                    # steady-state rep: re-load inputs into the same-tag
                    # buffers (WAR deps serialize reps) so per-rep time
                    # includes the input HBM traffic.
                    for pr in range(n_pair):
                        load_pair(pr)
                if att_order is None:
                    # interleave kv-supers with attention supers
                    emit_kv_super(0)
                    sg_next = 0
                    for s in range(1, n_kv_sup):
                        while sg_next < n_sup and (2 * sg_next + 1) // 4 < s:
                            emit_attention_super(sg_next)
                            sg_next += 1
                        emit_kv_super(s)
                    while sg_next < n_sup:
                        emit_attention_super(sg_next)
                        sg_next += 1
                else:
                    done_kv = 0
                    for sg in att_order:
                        need = (2 * sg + 1) // 4 + 1
                        while done_kv < need:
                            emit_kv_super(done_kv)
                            done_kv += 1
                        emit_attention_super(sg)
    nc.compile()
    return nc


def make_core_inputs(xT_b, wq_b, wkv_b, bq_c, bkv_c, identf, identb, h, T):
    """Per-core input dict. xT_b: [C, T] bf16 for this core's batch."""
    n_sup = T // SUP
    perm = chunk_perm(h)
    xq = np.ascontiguousarray(
        xT_b.reshape(C, n_sup, 4, KC)[:, :, perm, :].reshape(C, T))
    mask = np.zeros((128, 2, SUP), dtype=ml_dtypes.bfloat16)
    p = np.arange(128)[:, None]
    col = np.arange(SUP)[None, :]
    qrel = np.asarray(perm)[col // KC] * KC + col % KC   # global query offset
    for m in range(2):
        kvrel = (2 * m + h) * KC + p                     # global kv offset
        mask[:, m, :] = (kvrel <= qrel)
    return {"xq": xq, "wq": wq_b, "wkv": wkv_b,
            "bq": bq_c, "bkv": bkv_c, "mask": mask,
            "identf": identf, "identb": identb}


def prep_inputs(x, Wq, bq, Wk, bk, Wv, bv, T):
    xT = np.ascontiguousarray(
        np.transpose(np.asarray(x, np.float32), (0, 2, 1))).astype(ml_dtypes.bfloat16)
    wq_b = np.asarray(Wq, np.float32).astype(ml_dtypes.bfloat16)
    wkv_b = np.concatenate([np.asarray(Wk, np.float32),
                            np.asarray(Wv, np.float32)], 1).astype(ml_dtypes.bfloat16)
    bq_c = np.asarray(bq, np.float32).reshape(H, 1).copy()
    bkv_c = np.concatenate([np.asarray(bk, np.float32),
                            np.asarray(bv, np.float32)]).reshape(2 * H, 1).copy()
    identf = np.eye(128, dtype=np.float32)
    identb = np.eye(128, dtype=ml_dtypes.bfloat16)
    n_b = xT.shape[0]
    return [make_core_inputs(xT[c // 2], wq_b, wkv_b, bq_c, bkv_c,
                             identf, identb, c % 2, T)
            for c in range(2 * n_b)]


def unpermute_rows(arr, h, T):
    """Undo the within-superblock query permutation on output rows."""
    n_sup = T // SUP
    perm = np.asarray(chunk_perm(h))
    a = arr.reshape(n_sup, 4, KC, -1)
    out = np.empty_like(a)
    out[:, perm, :, :] = a
    return out.reshape(T, -1)


def combine(results, T):
    n_b = len(results) // 2
    out = np.empty((n_b, T, H), np.float32)
    for b in range(n_b):
        r0 = unpermute_rows(results[2 * b]["out"], 0, T).astype(np.float64)
        r1 = unpermute_rows(results[2 * b + 1]["out"], 1, T).astype(np.float64)
        num = r0[:, :H] + r1[:, :H]
        den = r0[:, H:] + r1[:, H:]
        out[b] = (num / den).astype(np.float32)
    return out


_NC = None


def kernel(x, Wq, bq, Wk, bk, Wv, bv):
    global _NC
    T = np.asarray(x).shape[1]
    if _NC is None:
        _NC = build_nc(T)
    in_maps = prep_inputs(x, Wq, bq, Wk, bk, Wv, bv, T)
    res = run_bass_kernel_spmd(_NC, in_maps, core_ids=list(range(8)))
    return combine(res.results, T)



# revision 11
# speedup vs baseline: 1.6308x; 1.6308x over previous
"""Causal single-head attention (B=4, T=4096, C=1024, H=64) on 8 TRN2 cores.

Sharding: core = 2*b + h  (b = batch, h = kv-parity).  Each core computes,
for ALL queries of its batch, the partial softmax numerator and denominator
over the kv chunks (128 rows each) whose chunk index has parity h; the host
combines  out = (num0 + num1) / (den0 + den1).  The host passes x[b].T with
each 512-query superblock's four 128-column chunks permuted parity-major
(chunks [h, 2+h, 1-h, 3-h]), so the kv-parity columns of every superblock
sit at positions 0:256.  Causal masks (host-built, per-core) and host-side
row unpermutation absorb the data-dependence; the per-core program is
identical across cores (SPMD) and perfectly load-balanced.

Precision/engine plan (validated vs the fp32 reference in numpy, 9.2e-3):
  - x, Wq, Wk, Wv shipped fp8e4 (plus a small bf16 slice of x for the
    first kv-super); all projections run as fp8 DoubleRow matmuls
    (0.5 cyc/col, contraction 2x128 per instr), fp32 PSUM accum, biases
    added on DVE.
  - S^T = k8.T @ q8 via DoubleRow with a zero second q-plane (q8 layout
    [64, sup, 2, 512] with plane 1 = zeros), so each 128-kv chunk costs
    256 cycles instead of 512.
  - p = exp(S*scale) on ACT, written directly as fp8 (bf16 for query
    superblock 0, whose rows dominate the error budget); diagonal-pair
    mask applied as a 0/1 multiply.
  - PV: one DoubleRow matmul per kv-chunk PAIR: lhsT = vaug8 [128,2,65]
    (V rows + ones column -> denominator row 64 for free), rhs = p
    [128,2,512].  Superblock 0 uses the bf16 path (vaug_lo from bf16 x).
  - Output written transposed ([65, 512] per superblock); host divides
    num/den and unpermutes.
"""

import numpy as np
import ml_dtypes

import concourse.bass as bass
import concourse.bacc as bacc
import concourse.tile as tile
from concourse import mybir
from concourse.bass_utils import run_bass_kernel_spmd

F32 = mybir.dt.float32
BF16 = mybir.dt.bfloat16
F8 = mybir.dt.float8e4
NP_F8 = ml_dtypes.float8_e4m3

B = 4
C = 1024
H = 64
SUP = 512          # query superblock width
KC = 128           # kv chunk
CC = C // 128      # contraction chunks (8)


def chunk_perm(h):
    """Within-superblock 128-col chunk order: parity-h chunks first."""
    return [h, 2 + h, 1 - h, 3 - h]


def build_nc(T=4096, reps=1, debug=False):
    n_sup = T // SUP           # 8
    n_pair = n_sup // 2        # 4 kv-supers (each: 4 parity chunks)
    n_chunk = 4 * n_pair       # 16 parity kv chunks
    scale = float(C) ** -0.5
    DR = mybir.MatmulPerfMode.DoubleRow
    EXP = mybir.ActivationFunctionType.Exp

    nc = bacc.Bacc(None, target_bir_lowering=False)
    x8_d = nc.dram_tensor("x8", [128, n_sup, CC // 2, 1024], F8,
                          kind="ExternalInput")
    xb0_d = nc.dram_tensor("xb0", [128, CC, 256], BF16, kind="ExternalInput")
    wq8_d = nc.dram_tensor("wq8", [128, CC, H], F8, kind="ExternalInput")
    wk8_d = nc.dram_tensor("wk8", [128, CC, H], F8, kind="ExternalInput")
    wv8_d = nc.dram_tensor("wv8", [128, CC, H], F8, kind="ExternalInput")
    wvb_d = nc.dram_tensor("wvb", [128, CC, H], BF16, kind="ExternalInput")
    bq_d = nc.dram_tensor("bq", [H, 1], F32, kind="ExternalInput")
    bk_d = nc.dram_tensor("bk", [H, 1], F32, kind="ExternalInput")
    bv_d = nc.dram_tensor("bv", [H, 1], F32, kind="ExternalInput")
    mask8_d = nc.dram_tensor("mask8", [128, 2, SUP], F8, kind="ExternalInput")
    maskb_d = nc.dram_tensor("maskb", [128, 2, SUP], BF16, kind="ExternalInput")
    idb_d = nc.dram_tensor("identb", [128, 128], BF16, kind="ExternalInput")
    out_d = nc.dram_tensor("out", [n_sup, H + 1, SUP], F32,
                           kind="ExternalOutput")
    if debug:
        dq8_d = nc.dram_tensor("dq8", [H, n_sup, 2, SUP], F8,
                               kind="ExternalOutput")
        dk8_d = nc.dram_tensor("dk8", [H, 17, KC], F8, kind="ExternalOutput")
        dpt_d = nc.dram_tensor("dpt", [128, 2, SUP], F8, kind="ExternalOutput")
        dptb_d = nc.dram_tensor("dptb", [128, 2, SUP], BF16,
                                kind="ExternalOutput")
        dvaug_d = nc.dram_tensor("dvaug", [128, 2, 80], F8,
                                 kind="ExternalOutput")

    with tile.TileContext(nc) as tc:
        with (
            tc.tile_pool(name="consts", bufs=1) as consts,
            tc.tile_pool(name="pers", bufs=1) as pers,
            tc.tile_pool(name="kvv", bufs=2) as kvvp,
            tc.tile_pool(name="pt", bufs=3) as ptp,
            tc.tile_pool(name="ptb", bufs=1) as ptbp,
            tc.tile_pool(name="ot", bufs=2) as otsb,
            tc.tile_pool(name="proj", bufs=2, space="PSUM") as projp,
            tc.tile_pool(name="spsum", bufs=2, space="PSUM") as sp,
            tc.tile_pool(name="otp", bufs=1, space="PSUM") as otp,
            tc.tile_pool(name="tpp", bufs=1, space="PSUM") as tpp,
        ):
            # ---- persistent SBUF state (same buffers every rep) ----
            # q8: plane 1 of every superblock window is a zero plane so the
            # S DoubleRow matmuls contract only the real 64-deep q plane.
            q8 = pers.tile([H, n_sup, 2, SUP], F8, tag="q8")
            # k8: chunk-major kT; one pad chunk at the end so lhsT windows
            # [kc:kc+2] stay in bounds (its values are multiplied by the
            # q8 zero plane).
            k8 = pers.tile([H, n_chunk + 1, KC], F8, tag="k8")
            # vaug8 padded to 80 cols: DoubleRow lhsT plane stride must be
            # even and 16B-aligned (s3_lw dual-fp8 restriction).  col 64 =
            # ones (denominator row), cols 65:80 = zeros (pad rows in PSUM).
            VP = 80
            vaug8 = {}
            for kp in range(n_chunk // 2):
                vaug8[kp] = pers.tile([128, 2, VP], F8, tag=f"vaug8_{kp}",
                                      name=f"vaug8_{kp}")
                nc.vector.memset(vaug8[kp][:, :, H:], 0.0)
                nc.vector.memset(vaug8[kp][:, :, H:H + 1], 1.0)
            vaug_lo = pers.tile([128, 2, H + 1], BF16, tag="vaug_lo")
            nc.vector.memset(vaug_lo[:, :, H:H + 1], 1.0)
            nc.vector.memset(q8[:, :, 1, :], 0.0)
            nc.vector.memset(k8[:, n_chunk, :], 0.0)

            st = {}  # per-rep input tiles, fixed tags

            def load_inputs(rep):
                eng = [nc.sync, nc.gpsimd]
                st["wq8"] = consts.tile([128, CC, H], F8, tag="wq8", name="wq8")
                nc.sync.dma_start(st["wq8"][:], wq8_d[:])
                st["wk8"] = consts.tile([128, CC, H], F8, tag="wk8", name="wk8")
                nc.sync.dma_start(st["wk8"][:], wk8_d[:])
                st["wv8"] = consts.tile([128, CC, H], F8, tag="wv8", name="wv8")
                nc.sync.dma_start(st["wv8"][:], wv8_d[:])
                st["wvb"] = consts.tile([128, CC, H], BF16, tag="wvb", name="wvb")
                nc.sync.dma_start(st["wvb"][:], wvb_d[:])
                for nm, d in (("bq", bq_d), ("bk", bk_d), ("bv", bv_d)):
                    st[nm] = consts.tile([H, 1], F32, tag=nm, name=nm)
                    nc.gpsimd.dma_start(st[nm][:], d[:])
                st["xb0"] = consts.tile([128, CC, 256], BF16, tag="xb0", name="xb0")
                nc.sync.dma_start(st["xb0"][:], xb0_d[:])
                # x8 tiles: supers 0,1 first (kv-super 0 + attention start)
                for s in range(n_sup):
                    for j in range(CC // 2):
                        t_ = consts.tile([128, 2, SUP], F8, tag=f"x8_{s}_{j}", name=f"x8_{s}_{j}")
                        eng[(s + j) % 2].dma_start(t_[:], x8_d[:, s, j, :])
                        st[s, j] = t_
                if rep == 0:
                    st["mask8"] = consts.tile([128, 2, SUP], F8, tag="mask8", name="mask8")
                    nc.gpsimd.dma_start(st["mask8"][:], mask8_d[:])
                    st["maskb"] = consts.tile([128, 2, SUP], BF16, tag="maskb", name="maskb")
                    nc.gpsimd.dma_start(st["maskb"][:], maskb_d[:])
                    st["idb"] = consts.tile([128, 128], BF16, tag="idb", name="idb")
                    nc.gpsimd.dma_start(st["idb"][:], idb_d[:])

            def emit_kv_super(s):
                # K projection: 4 parity chunks (4s..4s+4) from supers 2s,2s+1
                psk = projp.tile([H, SUP], F32, tag="proj")
                # the two 256-col chains must not interleave: start=True
                # marks the whole PSUM bank pending-zero, wiping the other
                # chain's partial accumulation
                for i in range(2):
                    for j in range(CC // 2):
                        nc.tensor.matmul(psk[:, i * 256:(i + 1) * 256],
                                         st["wk8"][:, 2 * j:2 * j + 2, :],
                                         st[2 * s + i, j][:, :, 0:256],
                                         start=(j == 0), stop=(j == CC // 2 - 1),
                                         perf_mode=DR)
                nc.vector.tensor_scalar_add(k8[:, 4 * s:4 * s + 4, :],
                                            psk[:], st["bk"][:])
                # V-hi projection (fp8): same cols
                psv = projp.tile([H, SUP], F32, tag="proj")
                for i in range(2):
                    for j in range(CC // 2):
                        nc.tensor.matmul(psv[:, i * 256:(i + 1) * 256],
                                         st["wv8"][:, 2 * j:2 * j + 2, :],
                                         st[2 * s + i, j][:, :, 0:256],
                                         start=(j == 0), stop=(j == CC // 2 - 1),
                                         perf_mode=DR)
                kvv = kvvp.tile([H, SUP], BF16, tag="kvv")
                nc.vector.tensor_scalar_add(kvv[:], psv[:], st["bv"][:])
                for c in range(4):
                    kp, i = (4 * s + c) // 2, (4 * s + c) % 2
                    tp = tpp.tile([128, H], BF16, tag="tp")
                    nc.tensor.transpose(tp[:], kvv[:, c * KC:(c + 1) * KC],
                                        st["idb"][0:H, 0:H])
                    nc.vector.tensor_copy(vaug8[kp][:, i, 0:H], tp[:])
                if s == 0:
                    # V-lo (bf16, chunks 0,1 only) for superblock 0's PV
                    psl = projp.tile([H, 256], F32, tag="proj")
                    for cc in range(CC):
                        nc.tensor.matmul(psl[:], st["wvb"][:, cc, :],
                                         st["xb0"][:, cc, :],
                                         start=(cc == 0), stop=(cc == CC - 1))
                    kvl = kvvp.tile([H, 256], BF16, tag="kvl")
                    nc.vector.tensor_scalar_add(kvl[:], psl[:], st["bv"][:])
                    for c in range(2):
                        tp = tpp.tile([128, H], BF16, tag="tp")
                        nc.tensor.transpose(tp[:], kvl[:, c * KC:(c + 1) * KC],
                                            st["idb"][0:H, 0:H])
                        nc.vector.tensor_copy(vaug_lo[:, c, 0:H], tp[:])

            def emit_attention_super(sg):
                # Q projection for this superblock -> q8 plane 0
                psq = projp.tile([H, SUP], F32, tag="proj")
                for j in range(CC // 2):
                    nc.tensor.matmul(psq[:], st["wq8"][:, 2 * j:2 * j + 2, :],
                                     st[sg, j][:],
                                     start=(j == 0), stop=(j == CC // 2 - 1),
                                     perf_mode=DR)
                nc.vector.tensor_scalar_add(q8[:, sg, 0, :], psq[:], st["bq"][:])

                fp8_path = sg > 0
                n_kp = sg + 1       # kv chunk pairs
                ot_ps = otp.tile([80, SUP], F32)
                pts = {}

                def emit_s_exp(kp):
                    s_ps = sp.tile([128, 2, SUP], F32)
                    for i in range(2):
                        kc = 2 * kp + i
                        nc.tensor.matmul(s_ps[:, i, :], k8[:, kc:kc + 2, :],
                                         q8[:, sg, :, :],
                                         start=True, stop=True, perf_mode=DR)
                    if fp8_path:
                        pt = ptp.tile([128, 2, SUP], F8)
                    else:
                        pt = ptbp.tile([128, 2, SUP], BF16)
                    nc.scalar.activation(pt[:], s_ps[:], EXP, scale=scale)
                    if kp == n_kp - 1:  # diagonal pair: apply causal mask
                        m = st["mask8"] if fp8_path else st["maskb"]
                        nc.vector.tensor_mul(pt[:], pt[:], m[:])
                    if debug and sg == 0 and kp == 0:
                        nc.sync.dma_start(dptb_d[:], pt[:])
                    if debug and sg == 1 and kp == 0:
                        nc.sync.dma_start(dpt_d[:], pt[:])
                    pts[kp] = pt

                def emit_pv(kp):
                    if fp8_path:
                        nc.tensor.matmul(ot_ps[:], vaug8[kp][:], pts[kp][:],
                                         start=(kp == 0), stop=(kp == n_kp - 1),
                                         perf_mode=DR)
                    else:
                        for i in range(2):
                            nc.tensor.matmul(ot_ps[0:H + 1, :], vaug_lo[:, i, :],
                                             pts[kp][:, i, :],
                                             start=(i == 0), stop=(i == 1))
                    del pts[kp]

                # software-pipelined: PV lags S/exp by one pair so the PE
                # never stalls ahead of ACT
                emit_s_exp(0)
                for kp in range(1, n_kp):
                    emit_s_exp(kp)
                    emit_pv(kp - 1)
                emit_pv(n_kp - 1)

                ot_s = otsb.tile([H + 1, SUP], F32)
                nc.vector.tensor_copy(ot_s[:], ot_ps[0:H + 1, :])
                nc.sync.dma_start(out_d[sg], ot_s[:])

            for _rep in range(reps):
                load_inputs(_rep)
                emit_kv_super(0)
                sg_next = 0
                for s in range(1, n_pair):
                    while sg_next < n_sup and (2 * sg_next + 1) // 4 < s:
                        emit_attention_super(sg_next)
                        sg_next += 1
                    emit_kv_super(s)
                while sg_next < n_sup:
                    emit_attention_super(sg_next)
                    sg_next += 1
                if debug and _rep == 0:
                    nc.sync.dma_start(dq8_d[:], q8[:])
                    nc.sync.dma_start(dk8_d[:], k8[:])
                    nc.sync.dma_start(dvaug_d[:], vaug8[0][:])
    nc.compile()
    return nc


def make_core_inputs(xT_b, w8s, wvb, biases, mask8, maskb, identb, h, T):
    """Per-core input dict. xT_b: [C, T] f32 for this core's batch."""
    n_sup = T // SUP
    perm = chunk_perm(h)
    xq = np.ascontiguousarray(
        xT_b.reshape(C, n_sup, 4, KC)[:, :, perm, :].reshape(C, T))
    x8 = np.ascontiguousarray(
        xq.astype(NP_F8).reshape(CC, 128, n_sup, SUP)
        .transpose(1, 2, 0, 3).reshape(128, n_sup, CC // 2, 1024))
    xb0 = np.ascontiguousarray(
        xq[:, 0:256].astype(ml_dtypes.bfloat16)
        .reshape(CC, 128, 256).transpose(1, 0, 2))
    mask = np.zeros((128, 2, SUP), dtype=np.float32)
    p = np.arange(128)[:, None]
    col = np.arange(SUP)[None, :]
    qrel = np.asarray(perm)[col // KC] * KC + col % KC   # global query offset
    for m in range(2):
        kvrel = (2 * m + h) * KC + p                     # global kv offset
        mask[:, m, :] = (kvrel <= qrel)
    wq8, wk8, wv8 = w8s
    bq, bk, bv = biases
    return {"x8": x8, "xb0": xb0,
            "wq8": wq8, "wk8": wk8, "wv8": wv8, "wvb": wvb,
            "bq": bq, "bk": bk, "bv": bv,
            "mask8": mask8 if mask8 is not None else mask.astype(NP_F8),
            "maskb": maskb if maskb is not None else
            mask.astype(ml_dtypes.bfloat16),
            "identb": identb}


def _wprep(W, dt):
    return np.ascontiguousarray(
        np.asarray(W, np.float32).astype(dt).reshape(CC, 128, H)
        .transpose(1, 0, 2))


def prep_inputs(x, Wq, bq, Wk, bk, Wv, bv, T):
    xT = np.ascontiguousarray(
        np.transpose(np.asarray(x, np.float32), (0, 2, 1)))
    w8s = tuple(_wprep(W, NP_F8) for W in (Wq, Wk, Wv))
    wvb = _wprep(Wv, ml_dtypes.bfloat16)
    biases = tuple(np.asarray(b, np.float32).reshape(H, 1).copy()
                   for b in (bq, bk, bv))
    identb = np.eye(128, dtype=ml_dtypes.bfloat16)
    n_b = xT.shape[0]
    out = []
    masks = {}
    for c in range(2 * n_b):
        h = c % 2
        m8, mb = masks.get(h, (None, None))
        d = make_core_inputs(xT[c // 2], w8s, wvb, biases, m8, mb,
                             identb, h, T)
        masks[h] = (d["mask8"], d["maskb"])
        out.append(d)
    return out


def unpermute_rows(arr, h, T):
    """Undo the within-superblock query permutation on output rows."""
    n_sup = T // SUP
    perm = np.asarray(chunk_perm(h))
    a = arr.reshape(n_sup, 4, KC, -1)
    out = np.empty_like(a)
    out[:, perm, :, :] = a
    return out.reshape(T, -1)


def combine(results, T):
    n_b = len(results) // 2
    n_sup = T // SUP
    out = np.empty((n_b, T, H), np.float32)
    for b in range(n_b):
        rs = []
        for h in range(2):
            r = results[2 * b + h]["out"]          # [n_sup, H+1, SUP]
            flat = r.transpose(0, 2, 1).reshape(T, H + 1)
            rs.append(unpermute_rows(flat, h, T).astype(np.float64))
        num = rs[0][:, :H] + rs[1][:, :H]
        den = rs[0][:, H:] + rs[1][:, H:]
        out[b] = (num / den).astype(np.float32)
    return out


_NC = None


def kernel(x, Wq, bq, Wk, bk, Wv, bv):
    global _NC
    T = np.asarray(x).shape[1]
    if _NC is None:
        _NC = build_nc(T)
    in_maps = prep_inputs(x, Wq, bq, Wk, bk, Wv, bv, T)
    res = run_bass_kernel_spmd(_NC, in_maps, core_ids=list(range(8)))
    return combine(res.results, T)


# revision 12
# speedup vs baseline: 2.3175x; 1.4211x over previous
"""Causal single-head attention (B=4, T=4096, C=1024, H=64) on 8 TRN2 cores.

Sharding: core = 2*b + h  (b = batch, h = kv-parity).  Each core computes,
for ALL queries of its batch, the partial softmax numerator and denominator
over the kv chunks (128 rows each) whose chunk index has parity h; the host
combines  out = (num0 + num1) / (den0 + den1).  The host passes x[b].T with
each 512-query superblock's four 128-column chunks permuted parity-major
(chunks [h, 2+h, 1-h, 3-h]), so the kv-parity columns of every superblock
sit at positions 0:256.  Causal masks (host-built, per-core) and host-side
row unpermutation absorb the data-dependence; the per-core program is
identical across cores (SPMD) and perfectly load-balanced.

Precision/engine plan (validated vs the fp32 reference in numpy, 9.2e-3):
  - x, Wq, Wk, Wv shipped fp8e4 (plus a small bf16 slice of x for the
    first kv-super); all projections run as fp8 DoubleRow matmuls
    (0.5 cyc/col, contraction 2x128 per instr), fp32 PSUM accum, biases
    added on DVE.
  - S^T = k8.T @ q8 via DoubleRow with a zero second q-plane (q8 layout
    [64, sup, 2, 512] with plane 1 = zeros), so each 128-kv chunk costs
    256 cycles instead of 512.
  - p = exp(S*scale) on ACT, written directly as fp8 (bf16 for query
    superblock 0, whose rows dominate the error budget); diagonal-pair
    mask applied as a 0/1 multiply.
  - PV: one DoubleRow matmul per kv-chunk PAIR: lhsT = vaug8 [128,2,65]
    (V rows + ones column -> denominator row 64 for free), rhs = p
    [128,2,512].  Superblock 0 uses the bf16 path (vaug_lo from bf16 x).
  - Output written transposed ([65, 512] per superblock); host divides
    num/den and unpermutes.
"""

import numpy as np
import ml_dtypes

import concourse.bass as bass
import concourse.bacc as bacc
import concourse.tile as tile
from concourse import mybir
from concourse.bass_utils import run_bass_kernel_spmd

F32 = mybir.dt.float32
BF16 = mybir.dt.bfloat16
F8 = mybir.dt.float8e4
NP_F8 = ml_dtypes.float8_e4m3

B = 4
C = 1024
H = 64
SUP = 512          # query superblock width
KC = 128           # kv chunk
CC = C // 128      # contraction chunks (8)


def chunk_perm(h):
    """Within-superblock 128-col chunk order: parity-h chunks first."""
    return [h, 2 + h, 1 - h, 3 - h]


def build_nc(T=4096, reps=1, debug=False):
    import os
    ABL_NO_RELOAD = bool(int(os.environ.get("ABL_NO_RELOAD", "0")))
    ABL_HALF_EXP = bool(int(os.environ.get("ABL_HALF_EXP", "0")))
    ABL_MASK_GPSIMD = bool(int(os.environ.get("ABL_MASK_GPSIMD", "0")))
    n_sup = T // SUP           # 8
    n_pair = n_sup // 2        # 4 kv-supers (each: 4 parity chunks)
    n_chunk = 4 * n_pair       # 16 parity kv chunks
    scale = float(C) ** -0.5
    DR = mybir.MatmulPerfMode.DoubleRow
    EXP = mybir.ActivationFunctionType.Exp

    nc = bacc.Bacc(None, target_bir_lowering=False)
    x8_d = nc.dram_tensor("x8", [128, n_sup, CC // 2, 1024], F8,
                          kind="ExternalInput")
    xb0_d = nc.dram_tensor("xb0", [128, CC, 256], BF16, kind="ExternalInput")
    wq8_d = nc.dram_tensor("wq8", [128, CC, H], F8, kind="ExternalInput")
    wk8_d = nc.dram_tensor("wk8", [128, CC, H], F8, kind="ExternalInput")
    wv8_d = nc.dram_tensor("wv8", [128, CC, H], F8, kind="ExternalInput")
    wvb_d = nc.dram_tensor("wvb", [128, CC, H], BF16, kind="ExternalInput")
    bq_d = nc.dram_tensor("bq", [H, 1], F32, kind="ExternalInput")
    bk_d = nc.dram_tensor("bk", [H, 1], F32, kind="ExternalInput")
    bv_d = nc.dram_tensor("bv", [H, 1], F32, kind="ExternalInput")
    mask8_d = nc.dram_tensor("mask8", [128, 2, SUP], F8, kind="ExternalInput")
    maskb_d = nc.dram_tensor("maskb", [128, 2, SUP], BF16, kind="ExternalInput")
    idb_d = nc.dram_tensor("identb", [128, 128], BF16, kind="ExternalInput")
    out_d = nc.dram_tensor("out", [n_sup, H + 1, SUP], F32,
                           kind="ExternalOutput")
    if debug:
        dq8_d = nc.dram_tensor("dq8", [H, n_sup, 2, SUP], F8,
                               kind="ExternalOutput")
        dk8_d = nc.dram_tensor("dk8", [H, 17, KC], F8, kind="ExternalOutput")
        dpt_d = nc.dram_tensor("dpt", [128, 2, SUP], F8, kind="ExternalOutput")
        dptb_d = nc.dram_tensor("dptb", [128, 2, SUP], BF16,
                                kind="ExternalOutput")
        dvaug_d = nc.dram_tensor("dvaug", [128, 2, 80], F8,
                                 kind="ExternalOutput")

    with tile.TileContext(nc) as tc:
        with (
            tc.tile_pool(name="consts", bufs=1) as consts,
            tc.tile_pool(name="pers", bufs=1) as pers,
            tc.tile_pool(name="kvv", bufs=2) as kvvp,
            tc.tile_pool(name="pt", bufs=3) as ptp,
            tc.tile_pool(name="ptb", bufs=1) as ptbp,
            tc.tile_pool(name="ot", bufs=2) as otsb,
            tc.tile_pool(name="proj", bufs=2, space="PSUM") as projp,
            tc.tile_pool(name="spsum", bufs=2, space="PSUM") as sp,
            tc.tile_pool(name="otp", bufs=1, space="PSUM") as otp,
            tc.tile_pool(name="tpp", bufs=1, space="PSUM") as tpp,
        ):
            # ---- persistent SBUF state (same buffers every rep) ----
            # q8: plane 1 of every superblock window is a zero plane so the
            # S DoubleRow matmuls contract only the real 64-deep q plane.
            q8 = pers.tile([H, n_sup, 2, SUP], F8, tag="q8")
            # k8: chunk-major kT; one pad chunk at the end so lhsT windows
            # [kc:kc+2] stay in bounds (its values are multiplied by the
            # q8 zero plane).
            k8 = pers.tile([H, n_chunk + 1, KC], F8, tag="k8")
            # vaug8 padded to 80 cols: DoubleRow lhsT plane stride must be
            # even and 16B-aligned (s3_lw dual-fp8 restriction).  col 64 =
            # ones (denominator row), cols 65:80 = zeros (pad rows in PSUM).
            VP = 80
            vaug8 = {}
            for kp in range(n_chunk // 2):
                vaug8[kp] = pers.tile([128, 2, VP], F8, tag=f"vaug8_{kp}",
                                      name=f"vaug8_{kp}")
                nc.vector.memset(vaug8[kp][:, :, H:], 0.0)
                nc.vector.memset(vaug8[kp][:, :, H:H + 1], 1.0)
            vaug_lo = pers.tile([128, 2, H + 1], BF16, tag="vaug_lo")
            nc.vector.memset(vaug_lo[:, :, H:H + 1], 1.0)
            nc.vector.memset(q8[:, :, 1, :], 0.0)
            nc.vector.memset(k8[:, n_chunk, :], 0.0)

            st = {}  # per-rep input tiles, fixed tags

            def load_inputs(rep):
                eng = [nc.sync, nc.gpsimd]
                st["wq8"] = consts.tile([128, CC, H], F8, tag="wq8", name="wq8")
                nc.sync.dma_start(st["wq8"][:], wq8_d[:])
                st["wk8"] = consts.tile([128, CC, H], F8, tag="wk8", name="wk8")
                nc.sync.dma_start(st["wk8"][:], wk8_d[:])
                st["wv8"] = consts.tile([128, CC, H], F8, tag="wv8", name="wv8")
                nc.sync.dma_start(st["wv8"][:], wv8_d[:])
                st["wvb"] = consts.tile([128, CC, H], BF16, tag="wvb", name="wvb")
                nc.sync.dma_start(st["wvb"][:], wvb_d[:])
                for nm, d in (("bq", bq_d), ("bk", bk_d), ("bv", bv_d)):
                    st[nm] = consts.tile([H, 1], F32, tag=nm, name=nm)
                    nc.gpsimd.dma_start(st[nm][:], d[:])
                st["xb0"] = consts.tile([128, CC, 256], BF16, tag="xb0", name="xb0")
                nc.sync.dma_start(st["xb0"][:], xb0_d[:])
                # x8 tiles: supers 0,1 first (kv-super 0 + attention start)
                for s in range(n_sup):
                    for j in range(CC // 2):
                        t_ = consts.tile([128, 2, SUP], F8, tag=f"x8_{s}_{j}", name=f"x8_{s}_{j}")
                        eng[(s + j) % 2].dma_start(t_[:], x8_d[:, s, j, :])
                        st[s, j] = t_
                if rep == 0:
                    st["mask8"] = consts.tile([128, 2, SUP], F8, tag="mask8", name="mask8")
                    nc.gpsimd.dma_start(st["mask8"][:], mask8_d[:])
                    st["maskb"] = consts.tile([128, 2, SUP], BF16, tag="maskb", name="maskb")
                    nc.gpsimd.dma_start(st["maskb"][:], maskb_d[:])
                    st["idb"] = consts.tile([128, 128], BF16, tag="idb", name="idb")
                    nc.gpsimd.dma_start(st["idb"][:], idb_d[:])

            def emit_kv_super(s):
                # K projection: 4 parity chunks (4s..4s+4) from supers 2s,2s+1
                psk = projp.tile([H, SUP], F32, tag="proj")
                # the two 256-col chains must not interleave: start=True
                # marks the whole PSUM bank pending-zero, wiping the other
                # chain's partial accumulation
                for i in range(2):
                    for j in range(CC // 2):
                        nc.tensor.matmul(psk[:, i * 256:(i + 1) * 256],
                                         st["wk8"][:, 2 * j:2 * j + 2, :],
                                         st[2 * s + i, j][:, :, 0:256],
                                         start=(j == 0), stop=(j == CC // 2 - 1),
                                         perf_mode=DR)
                nc.vector.tensor_scalar_add(k8[:, 4 * s:4 * s + 4, :],
                                            psk[:], st["bk"][:])
                # V-hi projection (fp8): same cols
                psv = projp.tile([H, SUP], F32, tag="proj")
                for i in range(2):
                    for j in range(CC // 2):
                        nc.tensor.matmul(psv[:, i * 256:(i + 1) * 256],
                                         st["wv8"][:, 2 * j:2 * j + 2, :],
                                         st[2 * s + i, j][:, :, 0:256],
                                         start=(j == 0), stop=(j == CC // 2 - 1),
                                         perf_mode=DR)
                kvv = kvvp.tile([H, SUP], BF16, tag="kvv")
                nc.vector.tensor_scalar_add(kvv[:], psv[:], st["bv"][:])
                for c in range(4):
                    kp, i = (4 * s + c) // 2, (4 * s + c) % 2
                    tp = tpp.tile([128, H], BF16, tag="tp")
                    nc.tensor.transpose(tp[:], kvv[:, c * KC:(c + 1) * KC],
                                        st["idb"][0:H, 0:H])
                    nc.vector.tensor_copy(vaug8[kp][:, i, 0:H], tp[:])
                if s == 0:
                    # V-lo (bf16, chunks 0,1 only) for superblock 0's PV
                    psl = projp.tile([H, 256], F32, tag="proj")
                    for cc in range(CC):
                        nc.tensor.matmul(psl[:], st["wvb"][:, cc, :],
                                         st["xb0"][:, cc, :],
                                         start=(cc == 0), stop=(cc == CC - 1))
                    kvl = kvvp.tile([H, 256], BF16, tag="kvl")
                    nc.vector.tensor_scalar_add(kvl[:], psl[:], st["bv"][:])
                    for c in range(2):
                        tp = tpp.tile([128, H], BF16, tag="tp")
                        nc.tensor.transpose(tp[:], kvl[:, c * KC:(c + 1) * KC],
                                            st["idb"][0:H, 0:H])
                        nc.vector.tensor_copy(vaug_lo[:, c, 0:H], tp[:])

            def emit_attention_super(sg):
                # Q projection for this superblock -> q8 plane 0
                psq = projp.tile([H, SUP], F32, tag="proj")
                for j in range(CC // 2):
                    nc.tensor.matmul(psq[:], st["wq8"][:, 2 * j:2 * j + 2, :],
                                     st[sg, j][:],
                                     start=(j == 0), stop=(j == CC // 2 - 1),
                                     perf_mode=DR)
                nc.vector.tensor_scalar_add(q8[:, sg, 0, :], psq[:], st["bq"][:])

                fp8_path = sg > 0
                n_kp = sg + 1       # kv chunk pairs
                ot_ps = otp.tile([80, SUP], F32)
                pts = {}

                def emit_s_exp(kp):
                    s_ps = sp.tile([128, 2, SUP], F32)
                    for i in range(2):
                        kc = 2 * kp + i
                        nc.tensor.matmul(s_ps[:, i, :], k8[:, kc:kc + 2, :],
                                         q8[:, sg, :, :],
                                         start=True, stop=True, perf_mode=DR)
                    if fp8_path:
                        pt = ptp.tile([128, 2, SUP], F8)
                    else:
                        pt = ptbp.tile([128, 2, SUP], BF16)
                    if ABL_HALF_EXP:
                        nc.scalar.activation(pt[:, 0, :], s_ps[:, 0, :], EXP,
                                             scale=scale)
                        nc.vector.tensor_copy(pt[:, 1, :], pt[:, 0, :])
                    else:
                        nc.scalar.activation(pt[:], s_ps[:], EXP, scale=scale)
                    if kp == n_kp - 1:  # diagonal pair: apply causal mask
                        m = st["mask8"] if fp8_path else st["maskb"]
                        eng_m = nc.gpsimd if ABL_MASK_GPSIMD else nc.vector
                        eng_m.tensor_mul(pt[:], pt[:], m[:])
                    if debug and sg == 0 and kp == 0:
                        nc.sync.dma_start(dptb_d[:], pt[:])
                    if debug and sg == 1 and kp == 0:
                        nc.sync.dma_start(dpt_d[:], pt[:])
                    pts[kp] = pt

                def emit_pv(kp):
                    if fp8_path:
                        nc.tensor.matmul(ot_ps[:], vaug8[kp][:], pts[kp][:],
                                         start=(kp == 0), stop=(kp == n_kp - 1),
                                         perf_mode=DR)
                    else:
                        for i in range(2):
                            nc.tensor.matmul(ot_ps[0:H + 1, :], vaug_lo[:, i, :],
                                             pts[kp][:, i, :],
                                             start=(i == 0), stop=(i == 1))
                    del pts[kp]

                # software-pipelined: PV lags S/exp by one pair so the PE
                # never stalls ahead of ACT
                emit_s_exp(0)
                for kp in range(1, n_kp):
                    emit_s_exp(kp)
                    emit_pv(kp - 1)
                emit_pv(n_kp - 1)

                ot_s = otsb.tile([H + 1, SUP], F32)
                nc.vector.tensor_copy(ot_s[:], ot_ps[0:H + 1, :])
                nc.sync.dma_start(out_d[sg], ot_s[:])

            for _rep in range(reps):
                if _rep == 0 or not ABL_NO_RELOAD:
                    load_inputs(_rep)
                emit_kv_super(0)
                sg_next = 0
                for s in range(1, n_pair):
                    while sg_next < n_sup and (2 * sg_next + 1) // 4 < s:
                        emit_attention_super(sg_next)
                        sg_next += 1
                    emit_kv_super(s)
                while sg_next < n_sup:
                    emit_attention_super(sg_next)
                    sg_next += 1
                if debug and _rep == 0:
                    nc.sync.dma_start(dq8_d[:], q8[:])
                    nc.sync.dma_start(dk8_d[:], k8[:])
                    nc.sync.dma_start(dvaug_d[:], vaug8[0][:])
    nc.compile()
    return nc


def make_core_inputs(xT_b, w8s, wvb, biases, mask8, maskb, identb, h, T):
    """Per-core input dict. xT_b: [C, T] f32 for this core's batch."""
    n_sup = T // SUP
    perm = chunk_perm(h)
    xq = np.ascontiguousarray(
        xT_b.reshape(C, n_sup, 4, KC)[:, :, perm, :].reshape(C, T))
    x8 = np.ascontiguousarray(
        xq.astype(NP_F8).reshape(CC, 128, n_sup, SUP)
        .transpose(1, 2, 0, 3).reshape(128, n_sup, CC // 2, 1024))
    xb0 = np.ascontiguousarray(
        xq[:, 0:256].astype(ml_dtypes.bfloat16)
        .reshape(CC, 128, 256).transpose(1, 0, 2))
    mask = np.zeros((128, 2, SUP), dtype=np.float32)
    p = np.arange(128)[:, None]
    col = np.arange(SUP)[None, :]
    qrel = np.asarray(perm)[col // KC] * KC + col % KC   # global query offset
    for m in range(2):
        kvrel = (2 * m + h) * KC + p                     # global kv offset
        mask[:, m, :] = (kvrel <= qrel)
    wq8, wk8, wv8 = w8s
    bq, bk, bv = biases
    return {"x8": x8, "xb0": xb0,
            "wq8": wq8, "wk8": wk8, "wv8": wv8, "wvb": wvb,
            "bq": bq, "bk": bk, "bv": bv,
            "mask8": mask8 if mask8 is not None else mask.astype(NP_F8),
            "maskb": maskb if maskb is not None else
            mask.astype(ml_dtypes.bfloat16),
            "identb": identb}


def _wprep(W, dt):
    return np.ascontiguousarray(
        np.asarray(W, np.float32).astype(dt).reshape(CC, 128, H)
        .transpose(1, 0, 2))


def prep_inputs(x, Wq, bq, Wk, bk, Wv, bv, T):
    xT = np.ascontiguousarray(
        np.transpose(np.asarray(x, np.float32), (0, 2, 1)))
    w8s = tuple(_wprep(W, NP_F8) for W in (Wq, Wk, Wv))
    wvb = _wprep(Wv, ml_dtypes.bfloat16)
    biases = tuple(np.asarray(b, np.float32).reshape(H, 1).copy()
                   for b in (bq, bk, bv))
    identb = np.eye(128, dtype=ml_dtypes.bfloat16)
    n_b = xT.shape[0]
    out = []
    masks = {}
    for c in range(2 * n_b):
        h = c % 2
        m8, mb = masks.get(h, (None, None))
        d = make_core_inputs(xT[c // 2], w8s, wvb, biases, m8, mb,
                             identb, h, T)
        masks[h] = (d["mask8"], d["maskb"])
        out.append(d)
    return out


def unpermute_rows(arr, h, T):
    """Undo the within-superblock query permutation on output rows."""
    n_sup = T // SUP
    perm = np.asarray(chunk_perm(h))
    a = arr.reshape(n_sup, 4, KC, -1)
    out = np.empty_like(a)
    out[:, perm, :, :] = a
    return out.reshape(T, -1)


def combine(results, T):
    n_b = len(results) // 2
    n_sup = T // SUP
    out = np.empty((n_b, T, H), np.float32)
    for b in range(n_b):
        rs = []
        for h in range(2):
            r = results[2 * b + h]["out"]          # [n_sup, H+1, SUP]
            flat = r.transpose(0, 2, 1).reshape(T, H + 1)
            rs.append(unpermute_rows(flat, h, T).astype(np.float64))
        num = rs[0][:, :H] + rs[1][:, :H]
        den = rs[0][:, H:] + rs[1][:, H:]
        out[b] = (num / den).astype(np.float32)
    return out


_NC = None


def kernel(x, Wq, bq, Wk, bk, Wv, bv):
    global _NC
    T = np.asarray(x).shape[1]
    if _NC is None:
        _NC = build_nc(T)
    in_maps = prep_inputs(x, Wq, bq, Wk, bk, Wv, bv, T)
    res = run_bass_kernel_spmd(_NC, in_maps, core_ids=list(range(8)))
    return combine(res.results, T)
